# revision 23
# baseline (speedup 1.0000x reference)
"""Trainium2 Bass kernel for nn_MultiHeadAttention_8040178778165 (v3).

Causal MHA (B=4, T=2048, C=1024, H=16) with RoPE, tensor-parallel over
heads: each of 8 NeuronCores owns 2 heads and computes a partial
y^T = Wout[rows]^T @ O for its heads; host sums the 8 f16 partials.

v3 over v2: cost-aware feeder (generators yield PE-ns; the attention
inner loop pulls exactly the per-kt Act/PE deficit so ScalarE never
outruns a starved PE), proj split into ml-halves so only the j=0 half
is force-drained at batch entry, startup DMAs split (wqk/xt(0,0) in
ci-halves, sin/cos in token-quarters) with partial-ci accumulation for
the first chunk, v-copies and 2/3 of ys copies on DVE (Act keeps the
exp stream), final-batch tail ys copies split 256-col across DVE+Act.
"""

import sys

sys.path.insert(0, "/opt/trn_rl_repo")

import numpy as np
import ml_dtypes

import concourse.bacc as bacc
import concourse.mybir as mybir
import concourse.tile as tile
from concourse.masks import make_identity
from concourse.bass_utils import run_bass_kernel_spmd

F32 = mybir.dt.float32
F16 = mybir.dt.float16
BF16 = mybir.dt.bfloat16
AX = mybir.AluOpType
EXP = mybir.ActivationFunctionType.Exp

B, T, C, H = 4, 2048, 1024, 16
HS = C // H            # 64
NT = B * T             # 8192
NCORES = 8
HPC = H // NCORES      # 2 heads per core
VST = 132              # vb stride per k-tile: [v_h0(64)|1|pad|v_h1(64)|1|pad]
SHUF = list(range(16, 32)) + list(range(16))  # swap 16-blocks in each quadrant

MM = 0.4167            # PE ns/col at full speed
ACT = 0.8333           # Act ns/col
ACT_FIX = 143.0        # Act per-instr PSUM overhead


def build_nc(debug=False):
    nc = bacc.Bacc()

    xT = nc.declare_dram_parameter("xT", [C, NT], BF16, isOutput=False)
    wqk = nc.declare_dram_parameter("wqk", [C, 256], BF16, isOutput=False)
    wv = nc.declare_dram_parameter("wv", [C, 128], BF16, isOutput=False)
    wo = nc.declare_dram_parameter("wo", [128, C], BF16, isOutput=False)
    bqk = nc.declare_dram_parameter("bqk", [128, 2], F32, isOutput=False)
    cosT = nc.declare_dram_parameter("cosT", [128, T], BF16, isOutput=False)
    sinP = nc.declare_dram_parameter("sinP", [128, T], BF16, isOutput=False)
    yT = nc.declare_dram_parameter("yT", [C, NT], F16, isOutput=True)

    with tile.TileContext(nc) as tc:
        with (
            tc.tile_pool(name="const", bufs=1) as cpool,
            tc.tile_pool(name="xin", bufs=6) as xpool,
            tc.tile_pool(name="qkv", bufs=3) as qkvpool,
            tc.tile_pool(name="rope", bufs=6) as rpool,
            tc.tile_pool(name="pt", bufs=10) as ptpool,
            tc.tile_pool(name="osc", bufs=2) as opool,
            tc.tile_pool(name="ao", bufs=2) as aopool,
            tc.tile_pool(name="ysb", bufs=3) as ypool,
            tc.tile_pool(name="small", bufs=8) as spool,
            tc.tile_pool(name="ps_sp", bufs=2, space="PSUM") as ps_sp,
            tc.tile_pool(name="ps_ot", bufs=2, space="PSUM") as ps_ot,
            tc.tile_pool(name="ps_mix", bufs=2, space="PSUM") as ps_mix,
        ):
            # ---- resident constants (DMAs emitted in the master schedule) ----
            wqk_sb = cpool.tile([128, 2048], BF16)
            bqk_sb = cpool.tile([128, 2], F32)
            cos_sb = cpool.tile([128, T], BF16)
            sin_sb = cpool.tile([128, T], BF16)
            wv_sb = cpool.tile([128, 1024], BF16)
            wo_sb = cpool.tile([128, C], BF16)
            ident_bf = cpool.tile([128, 128], BF16)
            make_identity(nc, ident_bf[:])
            # mneg[r, p] = -1e9 where r < p: mneg^T @ ident adds -1e9 to the
            # strict upper triangle (q < k) of a diagonal S tile pre-exp, so
            # exp() lands exact zeros and no post-exp mask is needed
            mneg = cpool.tile([128, 128], BF16)
            nc.gpsimd.memset(mneg[:], -1.0e9)
            nc.gpsimd.affine_select(
                out=mneg[:], in_=mneg[:], compare_op=AX.is_ge,
                fill=0.0, base=-1, pattern=[[1, 128]], channel_multiplier=-1)

            qkv_tiles = {}
            ys_count = [0]

            # ---------- feeder: FIFO of emission generators ----------
            class Feeder:
                def __init__(self):
                    self.q = []

                def push(self, gen, key=None):
                    self.q.append([key, gen])

                credit = 0.0

                def pull(self, budget):
                    # credit-based: over/under-shoot carries to the next pull
                    self.credit += budget
                    while self.q and self.credit > 0:
                        try:
                            self.credit -= next(self.q[0][1]) or 0.0
                        except StopIteration:
                            self.q.pop(0)
                    if not self.q and self.credit > 0:
                        self.credit = 0.0

                def drain_key(self, key):
                    for ent in list(self.q):
                        if ent[0] == key:
                            for _ in ent[1]:
                                pass
                            self.q.remove(ent)

                def drain(self):
                    while self.q:
                        try:
                            next(self.q[0][1])
                        except StopIteration:
                            self.q.pop(0)

            # ---------- x input DMAs ----------
            xts = {}

            def xdma(b, ml):
                tl = 512 * ml
                xt = xpool.tile([128, 4096], BF16, tag="xt", name=f"xt_{b}_{ml}")
                nc.sync.dma_start(
                    xt[:].rearrange("p (ci t) -> p ci t", ci=8, t=512),
                    xT[:, T * b + tl : T * b + tl + 512]
                    .rearrange("(ci p) t -> p ci t", ci=8, p=128))
                xts[(b, ml)] = xt

            def xdma_gen(b, mls=(0, 1, 2, 3)):
                for ml in mls:
                    xdma(b, ml)
                    yield 0.0

            # ---------- projection: qkv + rope for a half-batch ----------
            def proj_tiles(b):
                # emitted eagerly at push time: late emission of the memsets
                # gives them conservative consolidated waits that serialize
                # the next batch's exp stream behind them
                qT = qkvpool.tile([128, T], BF16, tag="qT", name=f"qT_{b}")
                kT = qkvpool.tile([128, T], BF16, tag="kT", name=f"kT_{b}")
                vb = qkvpool.tile([128, 16 * VST], BF16, tag="vb",
                                  name=f"vb_{b}")
                qkv_tiles[b] = (qT, kT, vb)
                nc.gpsimd.memset(vb[:, HS:16 * VST:VST], 1.0)
                nc.gpsimd.memset(vb[:, HS + 66:16 * VST:VST], 1.0)

            def proj_gen(b, mls):
                qT, kT, vb = qkv_tiles[b]
                for ml in mls:
                    tl = 512 * ml
                    xt = xts.pop((b, ml))
                    if b == 0 and ml == 0:
                        # startup: interleave q/k at ci-half granularity so PE
                        # can start on the first wqk/xt half-DMAs
                        psq = ps_mix.tile([128, 512], F32, tag="mix",
                                          name="ps_0_0_q")
                        psk = ps_mix.tile([128, 512], F32, tag="mix",
                                          name="ps_0_0_k")
                        for w, ps in ((0, psq), (1, psk)):
                            for ci in range(4):
                                nc.tensor.matmul(
                                    ps[:],
                                    wqk_sb[:, 256 * ci + 128 * w
                                           : 256 * ci + 128 * w + 128],
                                    xt[:, 512 * ci : 512 * ci + 512],
                                    start=(ci == 0), stop=False)
                                yield 213.0
                        for w, ps in ((0, psq), (1, psk)):
                            for ci in range(4, 8):
                                nc.tensor.matmul(
                                    ps[:],
                                    wqk_sb[:, 256 * ci + 128 * w
                                           : 256 * ci + 128 * w + 128],
                                    xt[:, 512 * ci : 512 * ci + 512],
                                    start=False, stop=(ci == 7))
                                yield 213.0
                        for w, ps, dest in ((0, psq, qT), (1, psk, kT)):
                            bias = bqk_sb[:, w : w + 1]
                            c1 = rpool.tile([128, 512], BF16, tag="c1")
                            nc.scalar.activation(
                                c1[:], ps[:],
                                mybir.ActivationFunctionType.Identity,
                                bias=bias, scale=1.0)
                            yield 0.0
                            u = rpool.tile([128, 512], BF16, tag="u")
                            nc.vector.tensor_tensor(
                                u[:], c1[:], sin_sb[:, tl : tl + 512],
                                op=AX.mult)
                            t1 = rpool.tile([128, 512], BF16, tag="t1")
                            nc.vector.tensor_tensor(
                                t1[:], c1[:], cos_sb[:, tl : tl + 512],
                                op=AX.mult)
                            yield 0.0
                            usw = rpool.tile([128, 512], BF16, tag="usw")
                            nc.vector.stream_shuffle(usw[:], u[:], SHUF)
                            nc.vector.tensor_tensor(
                                dest[:, tl : tl + 512], t1[:], usw[:],
                                op=AX.add)
                            yield 0.0
                    else:
                        for w, dest in ((0, qT), (1, kT)):
                            ps = ps_mix.tile([128, 512], F32, tag="mix",
                                             name=f"ps_{b}_{ml}_{w}")
                            for ci in range(8):
                                nc.tensor.matmul(
                                    ps[:],
                                    wqk_sb[:, 256 * ci + 128 * w
                                           : 256 * ci + 128 * w + 128],
                                    xt[:, 512 * ci : 512 * ci + 512],
                                    start=(ci == 0), stop=(ci == 7))
                                if ci % 2 == 1:
                                    yield 426.0
                            bias = bqk_sb[:, w : w + 1]
                            c1 = rpool.tile([128, 512], BF16, tag="c1")
                            if b == 0 and ml < 2:
                                nc.scalar.activation(
                                    c1[:], ps[:],
                                    mybir.ActivationFunctionType.Identity,
                                    bias=bias, scale=1.0)
                            else:
                                nc.vector.tensor_scalar_add(c1[:], ps[:], bias)
                            yield 0.0
                            u = rpool.tile([128, 512], BF16, tag="u")
                            nc.vector.tensor_tensor(
                                u[:], c1[:], sin_sb[:, tl : tl + 512],
                                op=AX.mult)
                            t1 = rpool.tile([128, 512], BF16, tag="t1")
                            nc.vector.tensor_tensor(
                                t1[:], c1[:], cos_sb[:, tl : tl + 512],
                                op=AX.mult)
                            yield 0.0
                            usw = rpool.tile([128, 512], BF16, tag="usw")
                            nc.vector.stream_shuffle(usw[:], u[:], SHUF)
                            nc.vector.tensor_tensor(
                                dest[:, tl : tl + 512], t1[:], usw[:],
                                op=AX.add)
                            yield 0.0
                    # batched v: 4 token-tiles accumulate into one 1-bank tile
                    # (4 regions; the first start=True matmul zeroes the bank)
                    vps = ps_mix.tile([128, 512], F32, tag="mix",
                                      name=f"vps_{b}_{ml}")
                    for ci in range(8):
                        for tk in range(4):
                            nc.tensor.matmul(
                                vps[:, 128 * tk : 128 * tk + 128],
                                xt[:, 512 * ci + 128 * tk
                                   : 512 * ci + 128 * tk + 128],
                                wv_sb[:, 128 * ci : 128 * ci + 128],
                                start=(ci == 0 and tk == 0), stop=(ci == 7))
                            if tk == 3:
                                yield 213.0
                    vcopy = nc.scalar.copy if b == 0 and ml < 2 \
                        else nc.vector.tensor_copy
                    vcopy(
                        vb[:, VST * 4 * ml : VST * 4 * ml + 4 * VST]
                        .rearrange("p (g x) -> p g x", g=4, x=VST)[:, :, 0:132]
                        .rearrange("p g (h i) -> p g h i", h=2, i=66)[:, :, :, 0:64],
                        vps[:].rearrange("p (g h i) -> p g h i", g=4, h=2, i=64))
                    yield 213.0

            # ---------- attention for one (batch, head, q-chunk) ----------
            def attn_group(b, h, j, osc, feeder, on_ot_done=None):
                qT, kT, vb = qkv_tiles[b]
                hr = slice(HS * h, HS * h + HS)
                qbase = 1024 * j
                nkt = 8 * j + 8
                ot0 = ps_ot.tile([128, 264], F32, tag="ot",
                                 name=f"ot0_{b}_{h}_{j}")
                ot1 = ps_ot.tile([128, 264], F32, tag="ot",
                                 name=f"ot1_{b}_{h}_{j}")
                otiles = (ot0, ot1)
                sps = {}

                def emit_S(kt):
                    diag = kt - 8 * j
                    o = max(0, diag * 128)
                    sp = ps_sp.tile([128, 1024], F32, tag="sp",
                                    name=f"sp_{b}_{h}_{j}_{kt}")
                    if diag >= 0:
                        # causal mask folded into the accumulation: -1e9 on
                        # the strict upper triangle of the diagonal tile
                        # (this matmul's start=True zeroes its whole bank)
                        nc.tensor.matmul(
                            sp[:, o : o + 128], mneg[:], ident_bf[:],
                            start=True, stop=False)
                    if o < 512:
                        nc.tensor.matmul(
                            sp[:, o:512],
                            kT[hr, 128 * kt : 128 * kt + 128],
                            qT[hr, qbase + o : qbase + 512],
                            start=(diag < 0), stop=True)
                    lo = max(o, 512)
                    nc.tensor.matmul(
                        sp[:, lo:1024],
                        kT[hr, 128 * kt : 128 * kt + 128],
                        qT[hr, qbase + lo : qbase + 1024],
                        start=(o < 512 or diag < 0), stop=True)
                    sps[kt] = (sp, o)
                    return 1024 - o

                def ot_epilogue(oi):
                    ot = otiles[oi]
                    rec = spool.tile([128, 4], F32, tag="rec")
                    nc.vector.reciprocal_approx_fast(rec[:], ot[:, HS : 264 : 66])
                    for si in range(4):
                        s = 4 * oi + si
                        tcol = 128 * (8 * j + s) + HS * h
                        nc.vector.tensor_scalar_mul(
                            osc[:, tcol : tcol + HS],
                            ot[:, 66 * si : 66 * si + HS],
                            rec[:, si : si + 1])

                emit_S(0)
                for kt in range(nkt):
                    diag = kt - 8 * j
                    o = max(0, diag) * 128
                    sc = emit_S(kt + 1) if kt + 1 < nkt else 0
                    sp, _ = sps.pop(kt)
                    pt = ptpool.tile([128, 1024], BF16, tag="pt",
                                     name=f"pt_{b}_{h}_{j}_{kt}")
                    nc.scalar.activation(
                        pt[:, o:1024], sp[:, o:1024], EXP,
                        scale=1.0 / np.sqrt(HS))
                    npv = 8 - max(0, diag)
                    act_ns = (1024 - o) * ACT + ACT_FIX
                    pe_ns = sc * MM + npv * 29.0
                    # deficit + a share of the global fill excess: spreading
                    # the excess here (between S(kt+1) and PV(kt) in PE order)
                    # never delays the next exp, unlike boundary drains
                    budget = act_ns - pe_ns + 280.0
                    if kt == 8 * j + 3:
                        pass  # pull after the epilogue below
                    elif kt >= nkt - 2:
                        # group tail: fill after PV so the next group's S
                        # isn't pushed out
                        pass
                    else:
                        feeder.pull(budget)
                    for s in range(max(0, diag), 8):
                        nc.tensor.matmul(
                            otiles[s // 4][:, 66 * (s % 4) : 66 * (s % 4) + 65],
                            pt[:, 128 * s : 128 * s + 128],
                            vb[:, VST * kt + 66 * h : VST * kt + 66 * h + 65],
                            start=(kt == 0 and s % 4 == 0),
                            stop=(s == diag))
                    if kt == 8 * j + 3:
                        ot_epilogue(0)   # ot0 regions all stopped; free early
                        if on_ot_done is not None:
                            on_ot_done(0)
                        feeder.pull(budget)
                    elif kt >= nkt - 2:
                        feeder.pull(budget)
                ot_epilogue(1)
                if on_ot_done is not None:
                    on_ot_done(1)
                feeder.pull(400.0)

            # ---------- output projection for one (batch, 512-chunk) ----------
            def out_gen(b, ml, osc, ao):
                for t in range(4 * ml, 4 * ml + 4):
                    tp = ps_mix.tile([128, 128], BF16, tag="mix",
                                     name=f"tp_{b}_{t}")
                    nc.tensor.transpose(tp[:], osc[:, 128 * t : 128 * t + 128],
                                        ident_bf[:])
                    nc.vector.tensor_copy(ao[:, 128 * t : 128 * t + 128], tp[:])
                    yield 53.0
                ys = ypool.tile([128, 4096], F16, tag="ys", name=f"ys_{b}_{ml}")
                tail = b == B - 1 and ml >= 2
                for ot in range(8):
                    yp = ps_mix.tile([128, 512], F32, tag="mix",
                                     name=f"yp_{b}_{ml}_{ot}")
                    nc.tensor.matmul(
                        yp[:], wo_sb[:, 128 * ot : 128 * ot + 128],
                        ao[:, 512 * ml : 512 * ml + 512],
                        start=True, stop=True)
                    i = ys_count[0]
                    ys_count[0] += 1
                    if tail:
                        # exp stream over: split each drain across DVE + Act
                        nc.vector.tensor_copy(
                            ys[:, 512 * ot : 512 * ot + 256], yp[:, 0:256])
                        nc.scalar.copy(
                            ys[:, 512 * ot + 256 : 512 * ot + 512],
                            yp[:, 256:512])
                    elif i % 5 == 4:
                        nc.scalar.copy(ys[:, 512 * ot : 512 * ot + 512], yp[:])
                    else:
                        nc.vector.tensor_copy(
                            ys[:, 512 * ot : 512 * ot + 512], yp[:])
                    if tail and ml == 3 and ot % 2 == 1 and ot < 7:
                        # very last chunk: quarter writebacks pipeline with
                        # the remaining copies
                        q = ot // 2
                        nc.sync.dma_start(
                            yT[256 * q : 256 * q + 256,
                               T * b + 512 * ml : T * b + 512 * ml + 512]
                            .rearrange("(ot p) t -> p ot t", ot=2, p=128),
                            ys[:, 1024 * q : 1024 * q + 1024]
                            .rearrange("p (ot t) -> p ot t", ot=2, t=512))
                    elif tail and ml == 2 and ot == 3:
                        nc.sync.dma_start(
                            yT[0:512, T * b + 512 * ml : T * b + 512 * ml + 512]
                            .rearrange("(ot p) t -> p ot t", ot=4, p=128),
                            ys[:, 0:2048]
                            .rearrange("p (ot t) -> p ot t", ot=4, t=512))
                    yield 213.0
                if tail and ml == 3:
                    nc.sync.dma_start(
                        yT[768:1024, T * b + 512 * ml : T * b + 512 * ml + 512]
                        .rearrange("(ot p) t -> p ot t", ot=2, p=128),
                        ys[:, 3072:4096]
                        .rearrange("p (ot t) -> p ot t", ot=2, t=512))
                elif tail and ml == 2:
                    nc.sync.dma_start(
                        yT[512:1024, T * b + 512 * ml : T * b + 512 * ml + 512]
                        .rearrange("(ot p) t -> p ot t", ot=4, p=128),
                        ys[:, 2048:4096]
                        .rearrange("p (ot t) -> p ot t", ot=4, t=512))
                else:
                    nc.sync.dma_start(
                        yT[:, T * b + 512 * ml : T * b + 512 * ml + 512]
                        .rearrange("(ot p) t -> p ot t", ot=8, p=128),
                        ys[:].rearrange("p (ot t) -> p ot t", ot=8, t=512))
                yield 0.0

            # ---------- master schedule ----------
            feeder = Feeder()
            # startup DMAs, split + ordered for earliest PE start (the DMA
            # pool serializes copies in emission order)
            nc.sync.dma_start(
                wqk_sb[:, 0:1024].rearrange("p (ci c) -> p ci c", ci=4, c=256),
                wqk[0:512].rearrange("(ci p) c -> p ci c", ci=4, p=128))
            xt00 = xpool.tile([128, 4096], BF16, tag="xt", name="xt_0_0")
            xts[(0, 0)] = xt00
            nc.sync.dma_start(
                xt00[:, 0:2048].rearrange("p (ci t) -> p ci t", ci=4, t=512),
                xT[0:512, 0:512].rearrange("(ci p) t -> p ci t", ci=4, p=128))
            nc.sync.dma_start(
                wqk_sb[:, 1024:2048].rearrange("p (ci c) -> p ci c", ci=4, c=256),
                wqk[512:1024].rearrange("(ci p) c -> p ci c", ci=4, p=128))
            nc.sync.dma_start(
                xt00[:, 2048:4096].rearrange("p (ci t) -> p ci t", ci=4, t=512),
                xT[512:1024, 0:512].rearrange("(ci p) t -> p ci t", ci=4, p=128))
            nc.sync.dma_start(bqk_sb[:], bqk[:])
            nc.sync.dma_start(sin_sb[:, 0:512], sinP[:, 0:512])
            nc.sync.dma_start(cos_sb[:, 0:512], cosT[:, 0:512])
            nc.sync.dma_start(
                wv_sb[:].rearrange("p (ci c) -> p ci c", ci=8, c=128),
                wv[:].rearrange("(ci p) c -> p ci c", ci=8, p=128))
            xdma(0, 1)
            nc.sync.dma_start(sin_sb[:, 512:2048], sinP[:, 512:2048])
            nc.sync.dma_start(cos_sb[:, 512:2048], cosT[:, 512:2048])
            xdma(0, 2)
            xdma(0, 3)
            nc.sync.dma_start(wo_sb[:], wo[:])
            # 0/1 lower-triangle mask (kept for reference; unused)
            trimask = cpool.tile([128, 128], BF16)
            nc.gpsimd.memset(trimask[:], 1.0)
            nc.gpsimd.affine_select(
                out=trimask[:], in_=trimask[:], compare_op=AX.is_ge,
                fill=0.0, base=0, pattern=[[1, 128]], channel_multiplier=-1)

            proj_tiles(0)
            g0a = proj_gen(0, (0, 1))
            for _ in g0a:     # batch-0 first half emitted inline pre-attention
                pass
            feeder.push(proj_gen(0, (2, 3)), key=("proj", 0, 1))
            for b in range(B):
                if b + 1 < B:
                    feeder.push(xdma_gen(b + 1), key=("xdma", b + 1))
                    feeder.drain_key(("xdma", b + 1))  # issue x DMAs up front
                    proj_tiles(b + 1)
                    feeder.push(proj_gen(b + 1, (0, 1)), key=("proj", b + 1, 0))
                if b >= 2:
                    # backstop: osc(b) reuses osc(b-2)'s buffer; its readers
                    # (out(b-2) transposes) must be emitted before the alloc
                    feeder.drain_key(("out", b - 2, 2))
                    feeder.drain_key(("out", b - 2, 3))
                osc = opool.tile([128, T], BF16, tag="osc", name=f"osc_{b}")
                ao = aopool.tile([128, T], BF16, tag="ao", name=f"ao_{b}")
                for j in (0, 1):
                    if j == 1:
                        feeder.drain_key(("proj", b, 1))
                        if b + 1 < B:
                            feeder.push(proj_gen(b + 1, (2, 3)),
                                        key=("proj", b + 1, 1))
                    for h in range(HPC):
                        if h == HPC - 1:
                            def cb(oi, b=b, j=j, osc=osc, ao=ao):
                                mlo = 2 * j + oi
                                feeder.push(out_gen(b, mlo, osc, ao),
                                            key=("out", b, mlo))
                            attn_group(b, h, j, osc, feeder, cb)
                        else:
                            attn_group(b, h, j, osc, feeder)
                if b + 1 < B:
                    feeder.drain_key(("proj", b + 1, 0))
                del qkv_tiles[b]
            feeder.drain()

    nc.compile()
    return nc


_NC_CACHE = None


def _get_nc():
    global _NC_CACHE
    if _NC_CACHE is None:
        _NC_CACHE = build_nc()
    return _NC_CACHE


def _rope_tables():
    half = HS // 2       # 32 rotation pairs per head
    thetas = 10000.0 ** (-np.arange(half, dtype=np.float64) / half)
    ang = np.arange(T, dtype=np.float64)[:, None] * thetas[None, :]   # (T, 32)
    sin = np.sin(ang).T.astype(np.float32)    # (32, T), row i = pair-freq i
    cos = np.cos(ang).T.astype(np.float32)
    # per 64-row head block, quadrant layout:
    #   rows  0-15: pairs 0-15 even channels  -> cos c0..15, sin +s0..15
    #   rows 16-31: pairs 0-15 odd channels   -> cos c0..15, sin -s0..15
    #   rows 32-47: pairs 16-31 even channels -> cos c16..31, sin +s16..31
    #   rows 48-63: pairs 16-31 odd channels  -> cos c16..31, sin -s16..31
    cos64 = np.concatenate([cos[0:16], cos[0:16], cos[16:32], cos[16:32]], axis=0)
    sin64 = np.concatenate([sin[0:16], -sin[0:16], sin[16:32], -sin[16:32]], axis=0)
    cos128 = np.tile(cos64, (2, 1)).astype(ml_dtypes.bfloat16)
    sin128 = np.tile(sin64, (2, 1)).astype(ml_dtypes.bfloat16)
    return cos128, sin128


# channel permutation per head matching the quadrant layout above
_PERM64 = np.concatenate([
    np.arange(0, 32, 2), np.arange(1, 32, 2),
    np.arange(32, 64, 2), np.arange(33, 64, 2)])


def _prep_inputs(x, Wqkv, bqkv, Wout):
    xTa = np.ascontiguousarray(x.reshape(NT, C).T.astype(ml_dtypes.bfloat16))
    cos128, sin128 = _rope_tables()

    in_maps = []
    for c in range(NCORES):
        h0, h1 = 2 * c, 2 * c + 1
        wq = np.concatenate(
            [Wqkv[:, HS * h0 : HS * h0 + HS][:, _PERM64],
             Wqkv[:, HS * h1 : HS * h1 + HS][:, _PERM64]], axis=1)
        wk = np.concatenate(
            [Wqkv[:, C + HS * h0 : C + HS * h0 + HS][:, _PERM64],
             Wqkv[:, C + HS * h1 : C + HS * h1 + HS][:, _PERM64]], axis=1)
        wqk_c = np.ascontiguousarray(
            np.concatenate([wq, wk], axis=1).astype(ml_dtypes.bfloat16))
        wv_c = np.ascontiguousarray(
            Wqkv[:, 2 * C + HS * h0 : 2 * C + HS * h0 + 2 * HS]
            .astype(ml_dtypes.bfloat16))
        pq = _PERM64
        bq = np.concatenate([bqkv[HS * h0 : HS * h0 + HS][pq],
                             bqkv[HS * h1 : HS * h1 + HS][pq]])
        bk = np.concatenate([bqkv[C + HS * h0 : C + HS * h0 + HS][pq],
                             bqkv[C + HS * h1 : C + HS * h1 + HS][pq]])
        bqk_c = np.ascontiguousarray(np.stack([bq, bk], axis=1).astype(np.float32))
        wo_c = np.ascontiguousarray(
            Wout[128 * c : 128 * c + 128, :].astype(ml_dtypes.bfloat16))
        in_maps.append({
            "xT": xTa,
            "wqk": wqk_c,
            "wv": wv_c,
            "wo": wo_c,
            "bqk": bqk_c,
            "cosT": cos128,
            "sinP": sin128,
        })
    return in_maps


def kernel(x, Wqkv, bqkv, Wout, bout, num_heads):
    x = np.asarray(x, dtype=np.float32)
    Wqkv = np.asarray(Wqkv, dtype=np.float32)
    bqkv = np.asarray(bqkv, dtype=np.float32)
    Wout = np.asarray(Wout, dtype=np.float32)
    bout = np.asarray(bout, dtype=np.float32)

    nc = _get_nc()
    in_maps = _prep_inputs(x, Wqkv, bqkv, Wout)
    res = run_bass_kernel_spmd(nc, in_maps, core_ids=list(range(NCORES)))

    acc = np.zeros((C, NT), dtype=np.float32)
    for c in range(NCORES):
        acc += res.results[c]["yT"].astype(np.float32)
    y = acc.T
    # bout plus the folded V-bias contribution bv @ Wout
    bv = bqkv[2 * C : 3 * C]
    y = y + (bout + bv @ Wout)[None, :].astype(np.float32)
    return y.reshape(B, T, C)


if __name__ == "__main__":
    rng = np.random.default_rng(0)
    x = rng.standard_normal((B, T, C), dtype=np.float32)
    Wqkv = rng.standard_normal((C, 3 * C), dtype=np.float32) / 32
    bqkv = rng.standard_normal((3 * C,), dtype=np.float32) * 0.01
    Wout = rng.standard_normal((C, C), dtype=np.float32) / 32
    bout = rng.standard_normal((C,), dtype=np.float32) * 0.01
    y = kernel(x=x, Wqkv=Wqkv, bqkv=bqkv, Wout=Wout, bout=bout, num_heads=H)
    print("kernel output", y.shape, y.dtype, np.abs(y).mean())
